# revision 4
# baseline (speedup 1.0000x reference)
"""Trainium2 Bass kernel for nn_Attention_43946105373274.

Causal multi-head attention with rotary embeddings applied to q, k and v.
B=2, N=2048, DIM=1024, H=16, DH=64, f32.

Sharding: 8 cores = (2 batches) x (4 head-groups of 4 heads).
Each core computes the qkv projection for its heads (w_qkv column-shard),
full causal attention for its heads, and a partial output projection
(w_out row-shard).  The host sums the 4 partials per batch and adds the
bias (the "all-reduce" of the output projection) — full inputs in, full
output out.

Per-core pipeline:
  0. x^T prepared on the host (layout repack + bf16 cast) so the device
     does only fast contiguous DMAs; cos/sin computed on device from the
     rotary freqs (PE transpose-first, range-reduced Sin spline, bf16).
  1. QKV projection = bf16 matmuls accumulating in f32 PSUM, producing
     q/k/v in transposed [channels, n] layout (two heads per tile).
     Rotary: rotate_half as an R-matrix matmul; combine on the vector
     engine in bf16 (2x mode); the rotate-half cast runs on gpsimd.
  2. Attention per head: S^T = (k-block)^T q via fp32-PSUM matmuls so the
     softmax probabilities come out pre-transposed for the AV matmul;
     exp on the scalar engine (scale folds in DH^-0.5), causal mask via
     gpsimd affine_select, AV accumulates with a ones-column appended to
     V so softmax row-sums fall out of the same matmul; normalization =
     reciprocal + PE ones-broadcast.
  3. Output projection in bf16; partials DMA'd out as bf16 and summed
     (plus bias) on the host.
"""

import sys
import numpy as np

if "/opt/trn_rl_repo" not in sys.path:
    sys.path.insert(0, "/opt/trn_rl_repo")

B, N, DIM, H, DH = 2, 2048, 1024, 16, 64
HPC = 4                     # heads per core
NCORES = 8
SCALE = DH ** -0.5
NT = N // 128               # 16 row tiles
KB = DIM // 128             # 8 contraction blocks
CW = 512                    # i-chunk width
NCH = N // CW               # 4 chunks
JW = 3 * HPC * DH           # 768 qkv columns per core

_CACHE = {}


def _build_program():
    import concourse.bass as bass  # noqa: F401
    import concourse.mybir as mybir
    import concourse.tile as tile
    from concourse import bacc

    F32 = mybir.dt.float32
    F32R = mybir.dt.float32r
    BF16 = mybir.dt.bfloat16
    AF = mybir.ActivationFunctionType
    OP = mybir.AluOpType

    nc = bacc.Bacc("TRN2", target_bir_lowering=False, debug=False,
                   num_devices=NCORES)

    xbT = nc.dram_tensor("xbT", [DIM, N], BF16, kind="ExternalInput")
    wqkv = nc.dram_tensor("wqkv", [DIM, JW], BF16, kind="ExternalInput")
    wout = nc.dram_tensor("wout", [HPC * DH, DIM], BF16, kind="ExternalInput")
    freqs = nc.dram_tensor("freqs", [N, DH], F32R, kind="ExternalInput")
    identR = nc.dram_tensor("identR", [128, 128], F32R, kind="ExternalInput")
    rmatD = nc.dram_tensor("rmatD", [128, 128], BF16, kind="ExternalInput")
    identB = nc.dram_tensor("identB", [128, 128], BF16, kind="ExternalInput")
    outD = nc.dram_tensor("out", [N, DIM], BF16, kind="ExternalOutput")

    MAGIC = 12582912.0          # 1.5 * 2**23: float32 round-to-nearest trick
    TWO_PI = float(2 * np.pi)

    with tile.TileContext(nc) as tc:
        with tc.tile_pool(name="pc", bufs=1) as pc, \
             tc.tile_pool(name="pw", bufs=1) as pw, \
             tc.tile_pool(name="pxT", bufs=1) as pxT, \
             tc.tile_pool(name="pqk", bufs=4) as pqk, \
             tc.tile_pool(name="pv", bufs=4) as pv, \
             tc.tile_pool(name="psb", bufs=2) as psb, \
             tc.tile_pool(name="ppt", bufs=6) as ppt, \
             tc.tile_pool(name="poT", bufs=2) as poT, \
             tc.tile_pool(name="pnm", bufs=2) as pnm, \
             tc.tile_pool(name="pout", bufs=1) as pout, \
             tc.tile_pool(name="psA", bufs=2, space="PSUM") as psA, \
             tc.tile_pool(name="ps5", bufs=3, space="PSUM") as ps5, \
             tc.tile_pool(name="psT", bufs=2, space="PSUM") as psT:

            # ---------------- DMAs: constants, weights, x^T ------------------
            ident = pc.tile([128, 128], F32R, tag="ident")
            nc.sync.dma_start(ident[:], identR[:])
            rmat = pc.tile([128, 128], BF16, tag="rmat")
            nc.sync.dma_start(rmat[:], rmatD[:])
            identb = pc.tile([128, 128], BF16, tag="identb")
            nc.sync.dma_start(identb[:], identB[:])
            ftile = pc.tile([128, NT * DH], F32R, tag="ftile")
            nc.sync.dma_start(
                ftile[:].rearrange("p (t d) -> p t d", t=NT),
                freqs[:].rearrange("(t p) d -> p t d", p=128))
            w_all = pw.tile([128, KB, JW], BF16, tag="w")
            nc.sync.dma_start(w_all[:], wqkv[:].rearrange("(k p) j -> p k j", p=128))
            xT = pxT.tile([128, KB, N], BF16, tag="xT")
            xbTv = xbT[:].rearrange("(k p) n -> p k n", p=128)
            for ch in range(NCH):
                nc.sync.dma_start(xT[:, :, ch * CW:(ch + 1) * CW],
                                  xbTv[:, :, ch * CW:(ch + 1) * CW])
            wo_all = pw.tile([128, 2, DIM], BF16, tag="wo")
            nc.sync.dma_start(wo_all[:], wout[:].rearrange("(k p) j -> p k j", p=128))

            # ---------------- small constants --------------------------------
            ones_f = pc.tile([128, 128], F32, tag="ones_f")
            nc.vector.memset(ones_f[:], 1.0)
            ones_r = pc.tile([1, 128], F32R, tag="ones_r")
            nc.vector.tensor_copy(ones_r[:], ones_f[0:1, :])

            # persistent tensors
            qT = [pqk.tile([128, N], BF16, tag="qk", name=f"qT{i}") for i in range(2)]
            kT = [pqk.tile([128, N], BF16, tag="qk", name=f"kT{i}") for i in range(2)]
            # V tiles: [128, 65] per (head, row-tile); col 64 = ones
            vt = [pv.tile([128, NT * (DH + 1)], BF16, tag="v", name=f"vt{h}", bufs=4)
                  for h in range(HPC)]
            for h in range(HPC):
                vv = vt[h][:].rearrange("p (t c) -> p t c", c=DH + 1)
                nc.vector.tensor_copy(vv[:, :, DH:DH + 1],
                                      ones_f[:, 0:NT].unsqueeze(2))
            oT = [poT.tile([128, N], BF16, tag="oT", name=f"oT{i}") for i in range(2)]

            # ---------------- trig prep: cos/sin in [d, n] bf16 --------------
            # transpose freqs tiles first -> arg [128=(sin:0-63, cos:64-127), n]
            argT = pout.tile([128, N], F32, tag="argT", bufs=1)
            for i in range(2):
                fps = psA.tile([128, 1024], F32R, tag="psA", name=f"fps{i}")
                for t in range(8):
                    tt = i * 8 + t
                    nc.tensor.transpose(fps[0:64, t * 128:(t + 1) * 128],
                                        ftile[:, tt * DH:(tt + 1) * DH],
                                        ident[:])
                nc.vector.tensor_copy(argT[0:64, i * 1024:(i + 1) * 1024],
                                      fps[0:64, :])
                nc.vector.tensor_scalar_add(argT[64:128, i * 1024:(i + 1) * 1024],
                                            fps[0:64, :], float(np.pi / 2))
            kt = pout.tile([128, N], F32, tag="kt", bufs=1)
            nc.vector.tensor_scalar(kt[:], argT[:], float(1.0 / TWO_PI), MAGIC,
                                    op0=OP.mult, op1=OP.add)
            nc.vector.tensor_scalar_sub(kt[:], kt[:], MAGIC)
            nc.vector.scalar_tensor_tensor(argT[:], kt[:], -TWO_PI, argT[:],
                                           op0=OP.mult, op1=OP.add)
            trigb = pc.tile([128, N], BF16, tag="trigb")
            nc.scalar.activation(trigb[:], argT[:], AF.Sin)
            sinb = pc.tile([128, N], BF16, tag="sinb")
            cosb = pc.tile([128, N], BF16, tag="cosb")
            nc.vector.tensor_copy(sinb[0:64, :], trigb[0:64, :])
            nc.vector.tensor_copy(sinb[64:128, :], trigb[0:64, :])
            nc.vector.tensor_copy(cosb[0:64, :], trigb[64:128, :])
            nc.vector.tensor_copy(cosb[64:128, :], trigb[64:128, :])

            # ---------------- phase 1: qkv projection + rotary ---------------
            for jt in (0, 2, 4, 1, 3, 5):
                for ch in range(NCH):
                    qps = ps5.tile([128, CW], F32, tag="ps512", name=f"qps{jt}_{ch}")
                    for kb in range(KB):
                        nc.tensor.matmul(
                            qps[:], w_all[:, kb, jt * 128:(jt + 1) * 128],
                            xT[:, kb, ch * CW:(ch + 1) * CW],
                            start=(kb == 0), stop=(kb == KB - 1))
                    t_sb = psb.tile([128, CW], BF16, tag="tsb", bufs=2)
                    nc.scalar.copy(t_sb[:], qps[:])
                    rps = ps5.tile([128, CW], F32, tag="ps512", name=f"rps{jt}_{ch}")
                    nc.tensor.matmul(rps[:], rmat[:], t_sb[:], start=True, stop=True)
                    r_sb = psb.tile([128, CW], BF16, tag="rsb", bufs=2)
                    nc.scalar.copy(r_sb[:], rps[:])
                    csl = cosb[:, ch * CW:(ch + 1) * CW]
                    ssl = sinb[:, ch * CW:(ch + 1) * CW]
                    tmp = psb.tile([128, CW], BF16, tag="tmp", bufs=2)
                    nc.vector.tensor_mul(tmp[:], t_sb[:], csl)
                    rs = psb.tile([128, CW], BF16, tag="rs2", bufs=2)
                    nc.vector.tensor_mul(rs[:], r_sb[:], ssl)
                    if jt < 4:  # q or k -> straight into qT/kT
                        dst = qT[jt] if jt < 2 else kT[jt - 2]
                        nc.vector.tensor_add(dst[:, ch * CW:(ch + 1) * CW],
                                             tmp[:], rs[:])
                    else:       # v -> rotate then transpose into V tiles
                        v_sb = psb.tile([128, CW], BF16, tag="vsb", bufs=2)
                        nc.vector.tensor_add(v_sb[:], tmp[:], rs[:])
                        pair = jt - 4
                        vps = psT.tile([128, CW], BF16, tag="pstr", bufs=1)
                        for rt in range(4):
                            nc.tensor.transpose(
                                vps[:, rt * 128:(rt + 1) * 128],
                                v_sb[:, rt * 128:(rt + 1) * 128],
                                identb[:])
                        vpsv = vps[:].rearrange("p (t hh d) -> p t hh d", t=4, hh=2)
                        for hh in range(2):
                            h = pair * 2 + hh
                            dstv = vt[h][:].rearrange("p (t c) -> p t c", c=DH + 1)[
                                :, ch * 4:(ch + 1) * 4, 0:DH]
                            nc.vector.tensor_copy(dstv, vpsv[:, :, hh, :])

            # ---------------- phase 2: attention per head --------------------
            for h in range(HPC):
                pair, hh = h // 2, h % 2
                qh = qT[pair][hh * 64:(hh + 1) * 64, :]
                kh = kT[pair][hh * 64:(hh + 1) * 64, :]

                def emit_norm(av_t, cc):
                    s_r = pnm.tile([1, CW], F32R, tag="s_r", bufs=2,
                                   name=f"s_r_{h}_{cc}")
                    nc.vector.tensor_copy(s_r[:], av_t[DH:DH + 1, :])
                    rbp = ps5.tile([64, CW], F32, tag="ps512", bufs=3,
                                   name=f"rbp_{h}_{cc}")
                    nc.tensor.matmul(rbp[:], ones_r[0:1, 0:64], s_r[:],
                                     start=True, stop=True)
                    rb = pnm.tile([64, CW], F32, tag="rb", bufs=2,
                                  name=f"rb_{h}_{cc}")
                    nc.vector.reciprocal_approx_fast(rb[:], rbp[:])
                    osl = oT[pair][hh * 64:(hh + 1) * 64, cc * CW:(cc + 1) * CW]
                    nc.vector.tensor_mul(osl, av_t[0:DH, :], rb[:])

                pending = None
                for c in range(NCH):
                    nj = 4 * c + 4          # j-blocks needed (causal)
                    av = ps5.tile([DH + 1, CW], F32, tag="ps512", bufs=3,
                                  name=f"av_{h}_{c}")
                    for grp in range(nj // 2):
                        j0 = grp * 2
                        sps = psA.tile([128, 1024], F32, tag="psA",
                                       name=f"sps_{h}_{c}_{grp}")
                        for g in range(2):
                            j = j0 + g
                            nc.tensor.matmul(
                                sps[:, g * 512:(g + 1) * 512],
                                kh[:, j * 128:(j + 1) * 128],
                                qh[:, c * CW:(c + 1) * CW],
                                start=True, stop=True)
                        pt = ppt.tile([128, 1024], BF16, tag="pt", bufs=6)
                        nc.scalar.activation(pt[:], sps[:], AF.Exp, scale=SCALE)
                        if j0 + 1 >= 4 * c:  # group touches the diagonal
                            w0 = min(CW, (j0 + 2 - 4 * c) * 128)
                            ptv = pt[:].rearrange("p (g i) -> p g i", g=2)[:, :, 0:w0]
                            nc.gpsimd.affine_select(
                                out=ptv, in_=ptv,
                                compare_op=OP.is_ge, fill=0.0,
                                base=c * CW - j0 * 128,
                                pattern=[[-128, 2], [1, w0]],
                                channel_multiplier=-1)
                        for g in range(2):
                            j = j0 + g
                            nc.tensor.matmul(av[:],
                                             vt[h][:, j * (DH + 1):(j + 1) * (DH + 1)],
                                             pt[:, g * 512:(g + 1) * 512],
                                             start=(j == 0), stop=(j == nj - 1))
                        if grp == 0 and pending is not None:
                            emit_norm(*pending)
                            pending = None
                    pending = (av, c)
                emit_norm(*pending)

            # ---------------- phase 3: output projection ---------------------
            for nt_i in range(NT):
                prj = psA.tile([128, DIM], F32, tag="psA", name=f"prj{nt_i}")
                for cb in range(2):
                    for mh in range(2):
                        nc.tensor.matmul(
                            prj[:, mh * 512:(mh + 1) * 512],
                            oT[cb][:, nt_i * 128:(nt_i + 1) * 128],
                            wo_all[:, cb, mh * 512:(mh + 1) * 512],
                            start=(cb == 0), stop=(cb == 1))
                ot = pout.tile([128, DIM], BF16, tag="osb", bufs=3)
                nc.vector.tensor_copy(ot[:, 0:512], prj[:, 0:512])
                nc.scalar.copy(ot[:, 512:1024], prj[:, 512:1024])
                nc.sync.dma_start(outD[nt_i * 128:(nt_i + 1) * 128, :], ot[:])

    nc.compile()
    return nc


def _get_program():
    if "nc" not in _CACHE:
        _CACHE["nc"] = _build_program()
    return _CACHE["nc"]


def _rot_lhsT():
    """lhsT for rot_half: out = lhsT.T @ tT = R @ tT, interleaved pairs."""
    R64 = np.zeros((64, 64), np.float32)
    for i in range(32):
        R64[2 * i, 2 * i + 1] = -1.0
        R64[2 * i + 1, 2 * i] = 1.0
    R = np.zeros((128, 128), np.float32)
    R[0:64, 0:64] = R64
    R[64:128, 64:128] = R64
    return np.ascontiguousarray(R.T)


def make_in_maps(x, rotary_pos_emb, w_qkv, w_out, b_out):
    x = np.asarray(x, np.float32)
    rotary_pos_emb = np.ascontiguousarray(np.asarray(rotary_pos_emb, np.float32))
    w_qkv = np.asarray(w_qkv, np.float32)
    w_out = np.asarray(w_out, np.float32)

    import ml_dtypes
    bf16 = ml_dtypes.bfloat16
    ident = np.eye(128, dtype=np.float32)
    identb = np.eye(128).astype(bf16)
    rmatT = _rot_lhsT()

    xT = [np.ascontiguousarray(x[b].T).astype(bf16) for b in range(B)]

    in_maps = []
    for c in range(NCORES):
        b = c // 4
        heads = [4 * (c % 4) + i for i in range(HPC)]
        # w_qkv column shard in j-tile order: q01,q23,k01,k23,v01,v23
        cols = []
        for t in range(3):            # q, k, v
            for h in heads:
                cols.append(w_qkv[:, t * H * DH + h * DH: t * H * DH + (h + 1) * DH])
        w_s = np.ascontiguousarray(np.concatenate(cols, axis=1))
        w_o = np.ascontiguousarray(
            np.concatenate([w_out[h * DH:(h + 1) * DH, :] for h in heads], axis=0))
        in_maps.append({
            "xbT": xT[b],
            "wqkv": w_s.astype(bf16),
            "wout": w_o.astype(bf16),
            "freqs": rotary_pos_emb,
            "identR": ident,
            "rmatD": rmatT.astype(bf16),
            "identB": identb,
        })
    return in_maps


def _gather(res, b_out):
    out = np.zeros((B, N, DIM), np.float32)
    for c in range(NCORES):
        out[c // 4] += np.asarray(res[c]["out"]).astype(np.float32)
    out += np.asarray(b_out, np.float32)[None, None, :]
    return out


def kernel(x, rotary_pos_emb, w_qkv, w_out, b_out):
    from concourse.bass_utils import run_bass_kernel_spmd

    nc = _get_program()
    in_maps = make_in_maps(x, rotary_pos_emb, w_qkv, w_out, b_out)
    res = run_bass_kernel_spmd(nc, in_maps, list(range(NCORES))).results
    return _gather(res, b_out)


# revision 5
# speedup vs baseline: 1.2597x; 1.2597x over previous
"""Trainium2 Bass kernel for nn_Attention_43946105373274.

Causal multi-head attention with rotary embeddings applied to q, k and v.
B=2, N=2048, DIM=1024, H=16, DH=64, f32.

Sharding: 8 cores = (2 batches) x (4 head-groups of 4 heads).
Each core computes the qkv projection for its heads (w_qkv column-shard),
full causal attention for its heads, and a partial output projection
(w_out row-shard).  The host sums the 4 partials per batch and adds the
bias (the "all-reduce" of the output projection) — full inputs in, full
output out.

Per-core pipeline:
  0. x^T prepared on the host (layout repack + bf16 cast) so the device
     does only fast contiguous DMAs; cos/sin computed on device from the
     rotary freqs (PE transpose-first, range-reduced Sin spline, bf16).
  1. QKV projection = bf16 matmuls accumulating in f32 PSUM, producing
     q/k/v in transposed [channels, n] layout (two heads per tile).
     Rotary: rotate_half as an R-matrix matmul; combine on the vector
     engine in bf16 (2x mode); the rotate-half cast runs on gpsimd.
  2. Attention per head: S^T = (k-block)^T q via fp32-PSUM matmuls so the
     softmax probabilities come out pre-transposed for the AV matmul;
     exp on the scalar engine (scale folds in DH^-0.5), causal mask via
     gpsimd affine_select, AV accumulates with a ones-column appended to
     V so softmax row-sums fall out of the same matmul; normalization =
     reciprocal + PE ones-broadcast.
  3. Output projection in bf16; partials DMA'd out as bf16 and summed
     (plus bias) on the host.
"""

import sys
import numpy as np

if "/opt/trn_rl_repo" not in sys.path:
    sys.path.insert(0, "/opt/trn_rl_repo")

B, N, DIM, H, DH = 2, 2048, 1024, 16, 64
HPC = 4                     # heads per core
NCORES = 8
SCALE = DH ** -0.5
NT = N // 128               # 16 row tiles
KB = DIM // 128             # 8 contraction blocks
CW = 512                    # i-chunk width
NCH = N // CW               # 4 chunks
JW = 3 * HPC * DH           # 768 qkv columns per core

_CACHE = {}


def _build_program():
    import concourse.bass as bass  # noqa: F401
    import concourse.mybir as mybir
    import concourse.tile as tile
    from concourse import bacc

    F32 = mybir.dt.float32
    F32R = mybir.dt.float32r
    BF16 = mybir.dt.bfloat16
    AF = mybir.ActivationFunctionType
    OP = mybir.AluOpType

    nc = bacc.Bacc("TRN2", target_bir_lowering=False, debug=False,
                   num_devices=NCORES)

    xbT = nc.dram_tensor("xbT", [DIM, N], BF16, kind="ExternalInput")
    wqkv = nc.dram_tensor("wqkv", [DIM, JW], BF16, kind="ExternalInput")
    wout = nc.dram_tensor("wout", [HPC * DH, DIM], BF16, kind="ExternalInput")
    freqs = nc.dram_tensor("freqs", [N, DH], F32R, kind="ExternalInput")
    identR = nc.dram_tensor("identR", [128, 128], F32R, kind="ExternalInput")
    rmatD = nc.dram_tensor("rmatD", [128, 128], BF16, kind="ExternalInput")
    identB = nc.dram_tensor("identB", [128, 128], BF16, kind="ExternalInput")
    outD = nc.dram_tensor("out", [N, DIM], BF16, kind="ExternalOutput")

    MAGIC = 12582912.0          # 1.5 * 2**23: float32 round-to-nearest trick
    TWO_PI = float(2 * np.pi)

    with tile.TileContext(nc) as tc:
        with tc.tile_pool(name="pc", bufs=1) as pc, \
             tc.tile_pool(name="pw", bufs=1) as pw, \
             tc.tile_pool(name="pxT", bufs=1) as pxT, \
             tc.tile_pool(name="pqk", bufs=4) as pqk, \
             tc.tile_pool(name="pv", bufs=4) as pv, \
             tc.tile_pool(name="psb", bufs=2) as psb, \
             tc.tile_pool(name="ppt", bufs=6) as ppt, \
             tc.tile_pool(name="poT", bufs=2) as poT, \
             tc.tile_pool(name="pnm", bufs=2) as pnm, \
             tc.tile_pool(name="pout", bufs=1) as pout, \
             tc.tile_pool(name="psA", bufs=2, space="PSUM") as psA, \
             tc.tile_pool(name="ps5", bufs=3, space="PSUM") as ps5, \
             tc.tile_pool(name="psT", bufs=2, space="PSUM") as psT:

            # ---------------- DMAs: constants, weights, x^T ------------------
            ident = pc.tile([128, 128], F32R, tag="ident")
            nc.sync.dma_start(ident[:], identR[:])
            rmat = pc.tile([128, 128], BF16, tag="rmat")
            nc.sync.dma_start(rmat[:], rmatD[:])
            identb = pc.tile([128, 128], BF16, tag="identb")
            nc.sync.dma_start(identb[:], identB[:])
            ftile = pc.tile([128, NT * DH], F32R, tag="ftile")
            nc.sync.dma_start(
                ftile[:].rearrange("p (t d) -> p t d", t=NT),
                freqs[:].rearrange("(t p) d -> p t d", p=128))
            w_all = pw.tile([128, KB, JW], BF16, tag="w")
            nc.sync.dma_start(w_all[:], wqkv[:].rearrange("(k p) j -> p k j", p=128))
            xT = pxT.tile([128, KB, N], BF16, tag="xT")
            xbTv = xbT[:].rearrange("(k p) n -> p k n", p=128)
            for ch in range(NCH):
                nc.sync.dma_start(xT[:, :, ch * CW:(ch + 1) * CW],
                                  xbTv[:, :, ch * CW:(ch + 1) * CW])
            wo_all = pw.tile([128, 2, DIM], BF16, tag="wo")
            nc.sync.dma_start(wo_all[:], wout[:].rearrange("(k p) j -> p k j", p=128))

            # ---------------- small constants --------------------------------
            ones_f = pc.tile([128, 128], F32, tag="ones_f")
            nc.vector.memset(ones_f[:], 1.0)
            ones_r = pc.tile([1, 128], F32R, tag="ones_r")
            nc.vector.tensor_copy(ones_r[:], ones_f[0:1, :])

            # persistent tensors
            qT = [pqk.tile([128, N], BF16, tag="qk", name=f"qT{i}") for i in range(2)]
            kT = [pqk.tile([128, N], BF16, tag="qk", name=f"kT{i}") for i in range(2)]
            # V tiles: [128, 65] per (head, row-tile); col 64 = ones
            vt = [pv.tile([128, NT * (DH + 1)], BF16, tag="v", name=f"vt{h}", bufs=4)
                  for h in range(HPC)]
            for h in range(HPC):
                vv = vt[h][:].rearrange("p (t c) -> p t c", c=DH + 1)
                nc.vector.tensor_copy(vv[:, :, DH:DH + 1],
                                      ones_f[:, 0:NT].unsqueeze(2))
            oT = [poT.tile([128, N], BF16, tag="oT", name=f"oT{i}") for i in range(2)]

            # ---------------- trig prep: cos/sin in [d, n] bf16 --------------
            # transpose freqs tiles first -> arg [128=(sin:0-63, cos:64-127), n]
            argT = pout.tile([128, N], F32, tag="argT", bufs=1)
            for i in range(2):
                fps = psA.tile([128, 1024], F32R, tag="psA", name=f"fps{i}")
                for t in range(8):
                    tt = i * 8 + t
                    nc.tensor.transpose(fps[0:64, t * 128:(t + 1) * 128],
                                        ftile[:, tt * DH:(tt + 1) * DH],
                                        ident[:])
                nc.vector.tensor_copy(argT[0:64, i * 1024:(i + 1) * 1024],
                                      fps[0:64, :])
                nc.vector.tensor_scalar_add(argT[64:128, i * 1024:(i + 1) * 1024],
                                            fps[0:64, :], float(np.pi / 2))
            kt = pout.tile([128, N], F32, tag="kt", bufs=1)
            nc.vector.tensor_scalar(kt[:], argT[:], float(1.0 / TWO_PI), MAGIC,
                                    op0=OP.mult, op1=OP.add)
            nc.vector.tensor_scalar_sub(kt[:], kt[:], MAGIC)
            nc.vector.scalar_tensor_tensor(argT[:], kt[:], -TWO_PI, argT[:],
                                           op0=OP.mult, op1=OP.add)
            trigb = pc.tile([128, N], BF16, tag="trigb")
            nc.scalar.activation(trigb[:], argT[:], AF.Sin)
            sinb = pc.tile([128, N], BF16, tag="sinb")
            cosb = pc.tile([128, N], BF16, tag="cosb")
            nc.vector.tensor_copy(sinb[0:64, :], trigb[0:64, :])
            nc.vector.tensor_copy(sinb[64:128, :], trigb[0:64, :])
            nc.vector.tensor_copy(cosb[0:64, :], trigb[64:128, :])
            nc.vector.tensor_copy(cosb[64:128, :], trigb[64:128, :])

            # ---------------- phase 1: qkv projection + rotary ---------------
            # Software-pipelined: the rotate-half matmul and combine of chain
            # n-1 are emitted after chain n's projection matmuls so the
            # tensor queue never blocks on the scalar cast round-trip.
            def finish_rotary(jt, ch, t_sb):
                rps = ps5.tile([128, CW], F32, tag="ps512", name=f"rps{jt}_{ch}")
                nc.tensor.matmul(rps[:], rmat[:], t_sb[:], start=True, stop=True)
                r_sb = psb.tile([128, CW], BF16, tag="rsb", bufs=4)
                nc.scalar.copy(r_sb[:], rps[:])
                csl = cosb[:, ch * CW:(ch + 1) * CW]
                ssl = sinb[:, ch * CW:(ch + 1) * CW]
                tmp = psb.tile([128, CW], BF16, tag="tmp", bufs=4)
                nc.vector.tensor_mul(tmp[:], t_sb[:], csl)
                rs = psb.tile([128, CW], BF16, tag="rs2", bufs=4)
                nc.vector.tensor_mul(rs[:], r_sb[:], ssl)
                if jt < 4:  # q or k -> straight into qT/kT
                    dst = qT[jt] if jt < 2 else kT[jt - 2]
                    nc.vector.tensor_add(dst[:, ch * CW:(ch + 1) * CW],
                                         tmp[:], rs[:])
                else:       # v -> rotate then transpose into V tiles
                    v_sb = psb.tile([128, CW], BF16, tag="vsb", bufs=2)
                    nc.vector.tensor_add(v_sb[:], tmp[:], rs[:])
                    pair = jt - 4
                    vps = psT.tile([128, CW], BF16, tag="pstr", bufs=1)
                    for rt in range(4):
                        nc.tensor.transpose(
                            vps[:, rt * 128:(rt + 1) * 128],
                            v_sb[:, rt * 128:(rt + 1) * 128],
                            identb[:])
                    vpsv = vps[:].rearrange("p (t hh d) -> p t hh d", t=4, hh=2)
                    for hh in range(2):
                        h = pair * 2 + hh
                        dstv = vt[h][:].rearrange("p (t c) -> p t c", c=DH + 1)[
                            :, ch * 4:(ch + 1) * 4, 0:DH]
                        nc.vector.tensor_copy(dstv, vpsv[:, :, hh, :])

            pending_rot = None
            for jt in (0, 2, 4, 1, 3, 5):
                for ch in range(NCH):
                    qps = ps5.tile([128, CW], F32, tag="ps512", name=f"qps{jt}_{ch}")
                    for kb in range(KB):
                        nc.tensor.matmul(
                            qps[:], w_all[:, kb, jt * 128:(jt + 1) * 128],
                            xT[:, kb, ch * CW:(ch + 1) * CW],
                            start=(kb == 0), stop=(kb == KB - 1))
                    if pending_rot is not None:
                        finish_rotary(*pending_rot)
                    t_sb = psb.tile([128, CW], BF16, tag="tsb", bufs=4)
                    nc.scalar.copy(t_sb[:], qps[:])
                    pending_rot = (jt, ch, t_sb)
            finish_rotary(*pending_rot)

            # ---------------- phase 2: attention per head --------------------
            for h in range(HPC):
                pair, hh = h // 2, h % 2
                qh = qT[pair][hh * 64:(hh + 1) * 64, :]
                kh = kT[pair][hh * 64:(hh + 1) * 64, :]

                def emit_norm(av_t, cc):
                    s_r = pnm.tile([1, CW], F32R, tag="s_r", bufs=2,
                                   name=f"s_r_{h}_{cc}")
                    nc.vector.tensor_copy(s_r[:], av_t[DH:DH + 1, :])
                    rbp = ps5.tile([64, CW], F32, tag="ps512", bufs=3,
                                   name=f"rbp_{h}_{cc}")
                    nc.tensor.matmul(rbp[:], ones_r[0:1, 0:64], s_r[:],
                                     start=True, stop=True)
                    rb = pnm.tile([64, CW], F32, tag="rb", bufs=2,
                                  name=f"rb_{h}_{cc}")
                    nc.vector.reciprocal_approx_fast(rb[:], rbp[:])
                    osl = oT[pair][hh * 64:(hh + 1) * 64, cc * CW:(cc + 1) * CW]
                    nc.vector.tensor_mul(osl, av_t[0:DH, :], rb[:])

                pending = None
                for c in range(NCH):
                    nj = 4 * c + 4          # j-blocks needed (causal)
                    av = ps5.tile([DH + 1, CW], F32, tag="ps512", bufs=3,
                                  name=f"av_{h}_{c}")
                    for grp in range(nj // 2):
                        j0 = grp * 2
                        sps = psA.tile([128, 1024], F32, tag="psA",
                                       name=f"sps_{h}_{c}_{grp}")
                        for g in range(2):
                            j = j0 + g
                            nc.tensor.matmul(
                                sps[:, g * 512:(g + 1) * 512],
                                kh[:, j * 128:(j + 1) * 128],
                                qh[:, c * CW:(c + 1) * CW],
                                start=True, stop=True)
                        pt = ppt.tile([128, 1024], BF16, tag="pt", bufs=6)
                        nc.scalar.activation(pt[:], sps[:], AF.Exp, scale=SCALE)
                        if j0 + 1 >= 4 * c:  # group touches the diagonal
                            w0 = min(CW, (j0 + 2 - 4 * c) * 128)
                            ptv = pt[:].rearrange("p (g i) -> p g i", g=2)[:, :, 0:w0]
                            nc.gpsimd.affine_select(
                                out=ptv, in_=ptv,
                                compare_op=OP.is_ge, fill=0.0,
                                base=c * CW - j0 * 128,
                                pattern=[[-128, 2], [1, w0]],
                                channel_multiplier=-1)
                        for g in range(2):
                            j = j0 + g
                            nc.tensor.matmul(av[:],
                                             vt[h][:, j * (DH + 1):(j + 1) * (DH + 1)],
                                             pt[:, g * 512:(g + 1) * 512],
                                             start=(j == 0), stop=(j == nj - 1))
                        if grp == 0 and pending is not None:
                            emit_norm(*pending)
                            pending = None
                    pending = (av, c)
                emit_norm(*pending)

            # ---------------- phase 3: output projection ---------------------
            for nt_i in range(NT):
                prj = psA.tile([128, DIM], F32, tag="psA", name=f"prj{nt_i}")
                for cb in range(2):
                    for mh in range(2):
                        nc.tensor.matmul(
                            prj[:, mh * 512:(mh + 1) * 512],
                            oT[cb][:, nt_i * 128:(nt_i + 1) * 128],
                            wo_all[:, cb, mh * 512:(mh + 1) * 512],
                            start=(cb == 0), stop=(cb == 1))
                ot = pout.tile([128, DIM], BF16, tag="osb", bufs=3)
                nc.vector.tensor_copy(ot[:, 0:512], prj[:, 0:512])
                nc.scalar.copy(ot[:, 512:1024], prj[:, 512:1024])
                nc.sync.dma_start(outD[nt_i * 128:(nt_i + 1) * 128, :], ot[:])

    nc.compile()
    return nc


def _get_program():
    if "nc" not in _CACHE:
        _CACHE["nc"] = _build_program()
    return _CACHE["nc"]


def _rot_lhsT():
    """lhsT for rot_half: out = lhsT.T @ tT = R @ tT, interleaved pairs."""
    R64 = np.zeros((64, 64), np.float32)
    for i in range(32):
        R64[2 * i, 2 * i + 1] = -1.0
        R64[2 * i + 1, 2 * i] = 1.0
    R = np.zeros((128, 128), np.float32)
    R[0:64, 0:64] = R64
    R[64:128, 64:128] = R64
    return np.ascontiguousarray(R.T)


def make_in_maps(x, rotary_pos_emb, w_qkv, w_out, b_out):
    x = np.asarray(x, np.float32)
    rotary_pos_emb = np.ascontiguousarray(np.asarray(rotary_pos_emb, np.float32))
    w_qkv = np.asarray(w_qkv, np.float32)
    w_out = np.asarray(w_out, np.float32)

    import ml_dtypes
    bf16 = ml_dtypes.bfloat16
    ident = np.eye(128, dtype=np.float32)
    identb = np.eye(128).astype(bf16)
    rmatT = _rot_lhsT()

    xT = [np.ascontiguousarray(x[b].T).astype(bf16) for b in range(B)]

    in_maps = []
    for c in range(NCORES):
        b = c // 4
        heads = [4 * (c % 4) + i for i in range(HPC)]
        # w_qkv column shard in j-tile order: q01,q23,k01,k23,v01,v23
        cols = []
        for t in range(3):            # q, k, v
            for h in heads:
                cols.append(w_qkv[:, t * H * DH + h * DH: t * H * DH + (h + 1) * DH])
        w_s = np.ascontiguousarray(np.concatenate(cols, axis=1))
        w_o = np.ascontiguousarray(
            np.concatenate([w_out[h * DH:(h + 1) * DH, :] for h in heads], axis=0))
        in_maps.append({
            "xbT": xT[b],
            "wqkv": w_s.astype(bf16),
            "wout": w_o.astype(bf16),
            "freqs": rotary_pos_emb,
            "identR": ident,
            "rmatD": rmatT.astype(bf16),
            "identB": identb,
        })
    return in_maps


def _gather(res, b_out):
    out = np.zeros((B, N, DIM), np.float32)
    for c in range(NCORES):
        out[c // 4] += np.asarray(res[c]["out"]).astype(np.float32)
    out += np.asarray(b_out, np.float32)[None, None, :]
    return out


def kernel(x, rotary_pos_emb, w_qkv, w_out, b_out):
    from concourse.bass_utils import run_bass_kernel_spmd

    nc = _get_program()
    in_maps = make_in_maps(x, rotary_pos_emb, w_qkv, w_out, b_out)
    res = run_bass_kernel_spmd(nc, in_maps, list(range(NCORES))).results
    return _gather(res, b_out)
